# revision 1
# baseline (speedup 1.0000x reference)
"""Llama decoder layer on 8 Trainium2 NeuronCores.

Hybrid sharding, all-bf16 matmuls:
  - QKV + attention: head-parallel (2 q heads + 1 kv head per core, full
    sequence) — statically balanced flash attention in transposed layout
    with ones-matmul denominators.
  - ONE AllToAll (2MB bf16 per rank) redistributes attention output from
    head-sharded to sequence-sharded. This replaces ALL ReduceScatters /
    AllGathers of the TP design (~950us of serial RDH collectives).
  - o_proj + residual + ln2 + MLP + final residual: sequence-parallel on
    each core's own 512 rows; o_proj/gate_up/down weights replicated in
    bf16 and streamed from HBM (no communication).
  - RMSNorm weights and the 1/sqrt(head_dim) q-scale are folded into the
    weights on the host; norms via ones-matmul sum-of-squares.

AllToAll layout: core i writes a2a_in rows [j*256,(j+1)*256) = its two
heads' aT columns [j*512,(j+1)*512); the collective gives core c rows
[i*256,(i+1)*256) = core i's shard c, i.e. af[a-block a] at rows a*128
(a = 2i+h) over the core's own seq columns [c*512,(c+1)*512).
"""
import os
import sys
sys.path.insert(0, "/opt/trn_rl_repo")

import numpy as np

import concourse.bass as bass
import concourse.mybir as mybir
import concourse.tile as tile
from concourse import bacc
from concourse.masks import make_identity

F32 = mybir.dt.float32
BF16 = mybir.dt.bfloat16
AF = mybir.ActivationFunctionType
ALU = mybir.AluOpType

NCORES = 8
SEQ = 4096
HID = 2048
NH = 16
NKV = 4
HD = 128
INTER = 5632
EPS = 1e-5
THETA = 10000.0

HB = HID // 128        # 16 hidden-dim 128-blocks
SCH = 512              # seq chunk
NCH = SEQ // SCH       # 8 chunks
OWN = 512              # rows owned per core (contiguous block c)
NM = INTER // 128      # 44 intermediate 128-blocks
ACOLS = NH // NCORES   # 2 q heads per core


def _build():
    nc = bacc.Bacc(None, num_devices=NCORES)

    xT_b = nc.dram_tensor("xT_b", [HID, SEQ], BF16, kind="ExternalInput")
    xT_own = nc.dram_tensor("xT_own", [HID, OWN], F32, kind="ExternalInput")
    cos_t = nc.dram_tensor("cos_t", [128, SEQ], F32, kind="ExternalInput")
    sin_t = nc.dram_tensor("sin_t", [128, SEQ], F32, kind="ExternalInput")
    w_qkv_s = nc.dram_tensor("w_qkv_s", [HID, (ACOLS + 2) * HD], BF16, kind="ExternalInput")
    # weights pre-tiled on host to the exact SBUF layout: one row-block of
    # 128 partitions x all contraction blocks side by side -> each load is
    # a single large contiguous 2D DMA, striped across 4 engine queues
    w_o_f = nc.dram_tensor("w_o_f", [HB, 128, HB * 128], BF16, kind="ExternalInput")
    w_gu_f = nc.dram_tensor("w_gu_f", [NM, 128, HB * 256], BF16, kind="ExternalInput")
    w_dn_f = nc.dram_tensor("w_dn_f", [HB, 128, NM * 128], BF16, kind="ExternalInput")
    out = nc.dram_tensor("out", [HID, OWN], F32, kind="ExternalOutput")

    dbg = {}
    if os.environ.get("KERNEL_DEBUG"):
        dbg["qT0"] = nc.dram_tensor("dbg_qT0", [128, SEQ], BF16, kind="ExternalOutput")
        dbg["kT"] = nc.dram_tensor("dbg_kT", [128, SEQ], BF16, kind="ExternalOutput")
        dbg["vN"] = nc.dram_tensor("dbg_vN", [128, SEQ], BF16, kind="ExternalOutput")
        dbg["aT0"] = nc.dram_tensor("dbg_aT0", [128, SEQ], BF16, kind="ExternalOutput")
        dbg["a2a"] = nc.dram_tensor("dbg_a2a", [NCORES * 2 * HD, OWN], BF16, kind="ExternalOutput")
        dbg["h1T"] = nc.dram_tensor("dbg_h1T", [128, HB * OWN], F32, kind="ExternalOutput")
        dbg["xm"] = nc.dram_tensor("dbg_xm", [128, HB * OWN], BF16, kind="ExternalOutput")
        dbg["hT"] = nc.dram_tensor("dbg_hT", [128, NM * OWN], BF16, kind="ExternalOutput")

    rg = [list(range(NCORES))]

    with tile.TileContext(nc) as tc:
        _emit(nc, tc, xT_b, xT_own, cos_t, sin_t,
              w_qkv_s, w_o_f, w_gu_f, w_dn_f, out, rg, dbg)
    nc.finalize()
    return nc


def _emit(nc, tc, xT_b, xT_own, cos_t, sin_t,
          w_qkv_s, w_o_f, w_gu_f, w_dn_f, out, rg, dbg={}):
    from contextlib import ExitStack
    es = ExitStack()
    # independent DMA-issue queues for weight streaming (SP/GpSimd/ACT)
    dmae = [nc.sync, nc.gpsimd, nc.scalar, nc.gpsimd]

    # ---------------- constants ----------------
    const = es.enter_context(tc.tile_pool(name="const", bufs=1))
    ident32 = const.tile([128, 128], F32, name="ident32")
    make_identity(nc, ident32)
    ident = const.tile([128, 128], BF16, name="ident")
    nc.vector.tensor_copy(ident[:], ident32[:])
    ones32 = const.tile([128, 1], F32, name="ones32")
    nc.vector.memset(ones32[:], 1.0)
    ones = const.tile([128, 1], BF16, name="ones")
    nc.vector.tensor_copy(ones[:], ones32[:])
    epsc = const.tile([128, 1], F32, name="epsc")
    nc.vector.memset(epsc[:], EPS)
    # causal masks for the 4 diagonal t-blocks of a 512-wide q chunk:
    # mask_j[p, col] = 1.0 if col - j*128 - p >= 0 else 0.0
    masks = []
    for j in range(4):
        m32 = const.tile([128, SCH], F32, name="m32scratch", tag="m32scratch")
        nc.vector.memset(m32[:], 1.0)
        nc.gpsimd.affine_select(
            out=m32[:], in_=m32[:], compare_op=ALU.is_ge,
            fill=0.0, base=-j * 128, channel_multiplier=-1, pattern=[[1, SCH]],
        )
        mj = const.tile([128, SCH], BF16, name=f"mask_{j}")
        nc.vector.tensor_copy(mj[:], m32[:])
        masks.append(mj)

    # ---------------- PSUM pools (8 banks) ----------------
    ps_mm = es.enter_context(tc.tile_pool(name="ps_mm", bufs=2, space="PSUM"))
    ps_s = es.enter_context(tc.tile_pool(name="ps_s", bufs=2, space="PSUM"))
    ps_o = es.enter_context(tc.tile_pool(name="ps_o", bufs=2, space="PSUM"))
    ps_row = es.enter_context(tc.tile_pool(name="ps_row", bufs=2, space="PSUM"))

    # ---------------- DRAM scratch ----------------
    dr_ag = es.enter_context(tc.tile_pool(name="dr_ag", bufs=1, space="DRAM"))
    a2a_in = dr_ag.tile([NCORES * ACOLS * HD, OWN], BF16, name="a2a_in")
    a2a_out = dr_ag.tile([NCORES * ACOLS * HD, OWN], BF16, name="a2a_out")

    wk = es.enter_context(tc.tile_pool(name="wk", bufs=4))

    ab = ExitStack()
    # persistent attention tensors (phases A+B)
    att = ab.enter_context(tc.tile_pool(name="att", bufs=1))
    qT = [att.tile([128, SEQ], BF16, name=f"qT{h}") for h in range(ACOLS)]
    kT = att.tile([128, SEQ], BF16, name="kT")
    vN = att.tile([128, SEQ], BF16, name="vN")
    aT = [att.tile([128, SEQ], BF16, name=f"aT{h}") for h in range(ACOLS)]

    # ============ Phase A: QKV + RMSNorm + RoPE (full seq) ============
    with tc.tile_pool(name="wqkv", bufs=1) as wqkvp, \
         tc.tile_pool(name="xbp", bufs=2) as xbp, \
         tc.tile_pool(name="trig", bufs=2) as trigp, \
         tc.tile_pool(name="aw", bufs=2) as aw, \
         tc.tile_pool(name="rw", bufs=2) as rw:

        wq_sb = wqkvp.tile([128, HB * (ACOLS + 2) * 128], BF16, name="wq_sb")
        QW = (ACOLS + 2) * 128
        for hb in range(HB):
            nc.sync.dma_start(wq_sb[:, hb * QW:(hb + 1) * QW],
                              w_qkv_s[hb * 128:(hb + 1) * 128, :])

        for sc in range(NCH):
            scol = slice(sc * SCH, (sc + 1) * SCH)
            cos_sb = trigp.tile([128, SCH], F32, name="cos_sb", tag="cos_sb")
            sin_sb = trigp.tile([128, SCH], F32, name="sin_sb", tag="sin_sb")
            nc.sync.dma_start(cos_sb[:], cos_t[:, scol])
            nc.sync.dma_start(sin_sb[:], sin_t[:, scol])
            xb = xbp.tile([128, HB * SCH], BF16, name="xb", tag="xb")
            for hb in range(HB):
                dmae[hb % 3].dma_start(xb[:, hb * SCH:(hb + 1) * SCH],
                                       xT_b[hb * 128:(hb + 1) * 128, scol])

            # sum of squares over hidden dim (per seq col) via ones-matmul
            ss_ps = ps_row.tile([1, SCH], F32, name="ss_ps", tag="row")
            for hb in range(HB):
                sq = aw.tile([128, SCH], BF16, name="sq", tag="sq")
                nc.vector.tensor_mul(sq[:], xb[:, hb * SCH:(hb + 1) * SCH],
                                     xb[:, hb * SCH:(hb + 1) * SCH])
                nc.tensor.matmul(ss_ps[:], ones[:], sq[:],
                                 start=(hb == 0), stop=(hb == HB - 1),
                                 skip_group_check=True)
            stdv = wk.tile([1, SCH], F32, name="stdv", tag="stdv")
            nc.scalar.activation(stdv[:], ss_ps[:], AF.Sqrt, scale=1.0 / HID,
                                 bias=epsc[0:1, :])
            rinv = wk.tile([1, SCH], F32, name="rinv", tag="rinv")
            nc.vector.reciprocal_approx_fast(rinv[:], stdv[:])
            rinv_bc = aw.tile([128, SCH], F32, name="rinv_bc", tag="rinv_bc")
            nc.gpsimd.partition_broadcast(rinv_bc[:], rinv[:])

            # qkv matmuls: col-block outer, hidden-block accumulation inner
            for cb in range(ACOLS + 2):
                qkv_ps = ps_mm.tile([128, SCH], F32, name="qkv_ps", tag="mm")
                for hb in range(HB):
                    nc.tensor.matmul(
                        qkv_ps[:],
                        wq_sb[:, (hb * (ACOLS + 2) + cb) * 128:
                              (hb * (ACOLS + 2) + cb + 1) * 128],
                        xb[:, hb * SCH:(hb + 1) * SCH],
                        start=(hb == 0), stop=(hb == HB - 1),
                        skip_group_check=True)
                raw = rw.tile([128, SCH], F32, name="raw", tag="raw")
                nc.vector.tensor_mul(raw[:], qkv_ps[:], rinv_bc[:])
                if cb < ACOLS + 1:
                    # rope into qT[cb] or kT: dst = raw*cos + swap(raw)*sin
                    dst = qT[cb] if cb < ACOLS else kT
                    swp = rw.tile([128, SCH], F32, name="swp", tag="swp")
                    nc.sync.dma_start(swp[0:64, :], raw[64:128, :])
                    nc.sync.dma_start(swp[64:128, :], raw[0:64, :])
                    t1 = rw.tile([128, SCH], F32, name="t1", tag="t1")
                    nc.vector.tensor_mul(t1[:], raw[:], cos_sb[:])
                    t2 = rw.tile([128, SCH], F32, name="t2", tag="t2")
                    nc.vector.tensor_mul(t2[:], swp[:], sin_sb[:])
                    nc.vector.tensor_add(dst[:, scol], t1[:], t2[:])
                else:
                    # V: transpose [d, s-chunk] -> natural [t, d] blocks
                    vt = rw.tile([128, SCH], BF16, name="vt", tag="vt")
                    nc.vector.tensor_copy(vt[:], raw[:])
                    for i in range(SCH // 128):
                        tp = ps_s.tile([128, 128], BF16, name="tp", tag="s")
                        nc.tensor.transpose(tp[:], vt[:, i * 128:(i + 1) * 128],
                                            ident[:])
                        nc.vector.tensor_copy(
                            vN[:, (sc * 4 + i) * 128:(sc * 4 + i + 1) * 128],
                            tp[:])

    # ============ Phase B: attention (2 heads, full seq) ============
    with tc.tile_pool(name="ew", bufs=4) as ew, \
         tc.tile_pool(name="atw", bufs=3) as atw:

        for qc in range(NCH):
            scol = slice(qc * SCH, (qc + 1) * SCH)
            ntb = 4 * qc + 4
            for h in range(ACOLS):
                o_ps = ps_o.tile([128, SCH], F32, name="o_ps", tag="o")
                den_ps = ps_row.tile([1, SCH], F32, name="den_ps", tag="row")
                # denominator: accumulate exp tiles on DVE (frees ~480
                # ones-matmuls off the tensor engine), one final
                # partition-sum matmul per (head, chunk)
                den_acc = atw.tile([128, SCH], F32, name="den_acc", tag="den_acc")
                for tb in range(ntb):
                    s_ps = ps_s.tile([128, SCH], F32, name="s_ps", tag="s")
                    nc.tensor.matmul(s_ps[:], kT[:, tb * 128:(tb + 1) * 128],
                                     qT[h][:, scol], start=True, stop=True,
                                     skip_group_check=True)
                    eT = ew.tile([128, SCH], BF16, name="eT", tag="eT")
                    nc.scalar.activation(eT[:], s_ps[:], AF.Exp)
                    j = tb - 4 * qc
                    if j >= 0:
                        eTm = ew.tile([128, SCH], BF16, name="eTm", tag="eTm")
                        nc.vector.tensor_mul(eTm[:], eT[:], masks[j][:])
                        eT = eTm
                    nc.tensor.matmul(o_ps[:], vN[:, tb * 128:(tb + 1) * 128],
                                     eT[:], start=(tb == 0), stop=(tb == ntb - 1),
                                     skip_group_check=True)
                    if tb == 0:
                        nc.vector.tensor_copy(den_acc[:], eT[:])
                    else:
                        nc.vector.tensor_add(den_acc[:], den_acc[:], eT[:])
                nc.tensor.matmul(den_ps[:], ones32[:], den_acc[:],
                                 start=True, stop=True, skip_group_check=True)
                dinv = wk.tile([1, SCH], F32, name="dinv", tag="dinv")
                nc.vector.reciprocal_approx_fast(dinv[:], den_ps[:])
                dinv_bc = atw.tile([128, SCH], F32, name="dinv_bc", tag="dinv_bc")
                nc.gpsimd.partition_broadcast(dinv_bc[:], dinv[:])
                nc.vector.tensor_mul(aT[h][:, scol], o_ps[:], dinv_bc[:])

        if dbg:
            nc.sync.dma_start(dbg["qT0"][:], qT[0][:])
            nc.sync.dma_start(dbg["kT"][:], kT[:])
            nc.sync.dma_start(dbg["vN"][:], vN[:])
            nc.sync.dma_start(dbg["aT0"][:], aT[0][:])

        # a2a_in rows [j*256,(j+1)*256) = both heads' aT cols [j*512,(j+1)*512)
        for j in range(NCORES):
            jcol = slice(j * OWN, (j + 1) * OWN)
            nc.sync.dma_start(a2a_in[j * 256:j * 256 + 128, :], aT[0][:, jcol])
            nc.sync.dma_start(a2a_in[j * 256 + 128:(j + 1) * 256, :], aT[1][:, jcol])

    nc.gpsimd.collective_compute(
        "AllToAll", ALU.bypass, replica_groups=rg,
        ins=[a2a_in[:].opt()], outs=[a2a_out[:].opt()])

    ab.close()

    # ============ Phase C: o_proj + residual + ln2 (own 512 rows) ======
    ch = ExitStack()
    hold = ch.enter_context(tc.tile_pool(name="hold", bufs=1))
    h1T = hold.tile([128, HB * OWN], F32, name="h1T")
    xm = hold.tile([128, HB * OWN], BF16, name="xm")

    with tc.tile_pool(name="afp", bufs=1) as afp, \
         tc.tile_pool(name="wop", bufs=2) as wop, \
         tc.tile_pool(name="xop", bufs=1) as xop, \
         tc.tile_pool(name="cw", bufs=2) as cw:

        af = afp.tile([128, HB * OWN], BF16, name="af")
        xot = xop.tile([128, HB * OWN], F32, name="xot")
        for a in range(HB):
            nc.sync.dma_start(af[:, a * OWN:(a + 1) * OWN],
                              a2a_out[a * 128:(a + 1) * 128, :])
            nc.sync.dma_start(xot[:, a * OWN:(a + 1) * OWN],
                              xT_own[a * 128:(a + 1) * 128, :])

        ss2_ps = ps_row.tile([1, OWN], F32, name="ss2_ps", tag="row")
        for n in range(HB):
            wo_t = wop.tile([128, HB * 128], BF16, name="wo_t", tag="wo_t")
            for i, eng in enumerate(dmae):
                W4 = HB * 128 // 4
                eng.dma_start(wo_t[:, i * W4:(i + 1) * W4],
                              w_o_f[n][:, i * W4:(i + 1) * W4])
            o_ps2 = ps_mm.tile([128, OWN], F32, name="o_ps2", tag="mm")
            for a in range(HB):
                nc.tensor.matmul(o_ps2[:], wo_t[:, a * 128:(a + 1) * 128],
                                 af[:, a * OWN:(a + 1) * OWN],
                                 start=(a == 0), stop=(a == HB - 1),
                                 skip_group_check=True)
            ncol = slice(n * OWN, (n + 1) * OWN)
            nc.vector.tensor_add(h1T[:, ncol], o_ps2[:], xot[:, ncol])
            sq2 = cw.tile([128, OWN], BF16, name="sq2", tag="sq2")
            nc.vector.tensor_mul(sq2[:], h1T[:, ncol], h1T[:, ncol])
            nc.tensor.matmul(ss2_ps[:], ones[:], sq2[:],
                             start=(n == 0), stop=(n == HB - 1),
                             skip_group_check=True)
        if dbg:
            nc.sync.dma_start(dbg["a2a"][:], a2a_out[:])
            for n in range(HB):
                ncol2 = slice(n * OWN, (n + 1) * OWN)
                h1c = cw.tile([128, OWN], F32, name="h1c", tag="h1c")
                nc.vector.tensor_copy(h1c[:], h1T[:, ncol2])
                nc.sync.dma_start(dbg["h1T"][:, ncol2], h1c[:])
        stdv2 = wk.tile([1, OWN], F32, name="stdv2", tag="stdv")
        nc.scalar.activation(stdv2[:], ss2_ps[:], AF.Sqrt, scale=1.0 / HID,
                             bias=epsc[0:1, :])
        rinv2 = wk.tile([1, OWN], F32, name="rinv2", tag="rinv")
        nc.vector.reciprocal_approx_fast(rinv2[:], stdv2[:])
        rinv2_bc = cw.tile([128, OWN], F32, name="rinv2_bc", tag="rinv2_bc")
        nc.gpsimd.partition_broadcast(rinv2_bc[:], rinv2[:])
        for n in range(HB):
            ncol = slice(n * OWN, (n + 1) * OWN)
            nc.vector.tensor_mul(xm[:, ncol], h1T[:, ncol], rinv2_bc[:])
        if dbg:
            nc.sync.dma_start(dbg["xm"][:], xm[:])

    # ============ Phase D: MLP (own 512 rows, streamed weights) ========
    with tc.tile_pool(name="hTp", bufs=1) as hTp, \
         tc.tile_pool(name="wgup", bufs=2) as wgup, \
         tc.tile_pool(name="wdnp", bufs=2) as wdnp, \
         tc.tile_pool(name="dw", bufs=2) as dw:

        hT = hTp.tile([128, NM * OWN], BF16, name="hT")
        for m in range(NM):
            wgu_t = wgup.tile([128, HB * 256], BF16, name="wgu_t", tag="wgu_t")
            for i, eng in enumerate(dmae):
                W4 = HB * 256 // 4
                eng.dma_start(wgu_t[:, i * W4:(i + 1) * W4],
                              w_gu_f[m][:, i * W4:(i + 1) * W4])
            g_ps = ps_mm.tile([128, OWN], F32, name="g_ps", tag="mm")
            for hb in range(HB):
                nc.tensor.matmul(g_ps[:], wgu_t[:, hb * 256:hb * 256 + 128],
                                 xm[:, hb * OWN:(hb + 1) * OWN],
                                 start=(hb == 0), stop=(hb == HB - 1),
                                 skip_group_check=True)
            u_ps = ps_mm.tile([128, OWN], F32, name="u_ps", tag="mm")
            for hb in range(HB):
                nc.tensor.matmul(u_ps[:], wgu_t[:, hb * 256 + 128:(hb + 1) * 256],
                                 xm[:, hb * OWN:(hb + 1) * OWN],
                                 start=(hb == 0), stop=(hb == HB - 1),
                                 skip_group_check=True)
            sg = dw.tile([128, OWN], F32, name="sg", tag="sg")
            if os.environ.get("KERNEL_SIM_SILU"):
                # CoreSim lacks Silu; x*sigmoid(x) is equivalent
                sig = dw.tile([128, OWN], F32, name="sig", tag="sig")
                nc.scalar.activation(sig[:], g_ps[:], AF.Sigmoid)
                nc.vector.tensor_mul(sg[:], g_ps[:], sig[:])
            else:
                nc.scalar.activation(sg[:], g_ps[:], AF.Silu)
            nc.vector.tensor_mul(hT[:, m * OWN:(m + 1) * OWN], sg[:], u_ps[:])

        if dbg:
            nc.sync.dma_start(dbg["hT"][:], hT[:])
        for n in range(HB):
            wdn_t = wdnp.tile([128, NM * 128], BF16, name="wdn_t", tag="wdn_t")
            for i, eng in enumerate(dmae):
                W4 = NM * 128 // 4
                eng.dma_start(wdn_t[:, i * W4:(i + 1) * W4],
                              w_dn_f[n][:, i * W4:(i + 1) * W4])
            dn_ps = ps_o.tile([128, OWN], F32, name="dn_ps", tag="o")
            for m in range(NM):
                nc.tensor.matmul(dn_ps[:], wdn_t[:, m * 128:(m + 1) * 128],
                                 hT[:, m * OWN:(m + 1) * OWN],
                                 start=(m == 0), stop=(m == NM - 1),
                                 skip_group_check=True)
            ot = dw.tile([128, OWN], F32, name="ot", tag="ot")
            nc.vector.tensor_add(ot[:], dn_ps[:], h1T[:, n * OWN:(n + 1) * OWN])
            nc.sync.dma_start(out[n * 128:(n + 1) * 128, :], ot[:])

    ch.close()
    es.close()


# ---------------- host side ----------------

_CACHE = {}


def _get_runner():
    if "runner" in _CACHE:
        return _CACHE["runner"]
    import jax
    from jax.sharding import Mesh, PartitionSpec
    from jax.experimental.shard_map import shard_map
    from concourse import bass2jax

    nc = _build()
    bass2jax.install_neuronx_cc_hook()

    in_names = []
    out_names = []
    out_avals = []
    zero_shapes = []
    for alloc in nc.m.functions[0].allocations:
        if not isinstance(alloc, mybir.MemoryLocationSet):
            continue
        name = alloc.memorylocations[0].name
        if alloc.kind == "ExternalInput":
            if nc.partition_id_tensor is None or name != nc.partition_id_tensor.name:
                in_names.append(name)
        elif alloc.kind == "ExternalOutput":
            out_names.append(name)
            shape = tuple(alloc.tensor_shape)
            dtype = mybir.dt.np(alloc.dtype)
            out_avals.append(jax.core.ShapedArray(shape, dtype))
            zero_shapes.append((shape, dtype))
    n_params = len(in_names)
    full_in_names = list(in_names) + list(out_names)
    if nc.partition_id_tensor is not None:
        full_in_names.append(nc.partition_id_tensor.name)
    import os
    donate = tuple(range(n_params, n_params + len(out_names)))
    if os.environ.get("KERNEL_NO_DONATE"):
        donate = ()

    def _body(*args):
        operands = list(args)
        if nc.partition_id_tensor is not None:
            operands.append(bass2jax.partition_id_tensor())
        outs = bass2jax._bass_exec_p.bind(
            *operands,
            out_avals=tuple(out_avals),
            in_names=tuple(full_in_names),
            out_names=tuple(out_names),
            lowering_input_output_aliases=(),
            sim_require_finite=True,
            sim_require_nnan=True,
            nc=nc,
        )
        return tuple(outs)

    devices = jax.devices()[:NCORES]
    mesh = Mesh(np.asarray(devices), ("core",))
    in_specs = (PartitionSpec("core"),) * (n_params + len(out_names))
    out_specs = (PartitionSpec("core"),) * len(out_names)
    sharded = jax.jit(
        shard_map(_body, mesh=mesh, in_specs=in_specs, out_specs=out_specs,
                  check_rep=False),
        donate_argnums=donate, keep_unused=True,
    )
    runner = dict(fn=sharded, in_names=in_names, out_names=out_names,
                  zero_shapes=zero_shapes, out_avals=out_avals)
    _CACHE["runner"] = runner
    _CACHE["nc"] = nc
    return runner


def _prep_inputs(positions, hidden_states, ln1_w, ln2_w, w_qkv, w_o, w_gate_up, w_down):
    """Build per-core input dicts (list of NCORES dicts, numpy)."""
    import ml_dtypes
    bf = ml_dtypes.bfloat16
    hs = np.asarray(hidden_states, dtype=np.float32)
    pos = np.asarray(positions, dtype=np.float64)
    ln1 = np.asarray(ln1_w, dtype=np.float32)
    ln2 = np.asarray(ln2_w, dtype=np.float32)
    wq = np.asarray(w_qkv, dtype=np.float32)
    wo = np.asarray(w_o, dtype=np.float32)
    wgu = np.asarray(w_gate_up, dtype=np.float32)
    wdn = np.asarray(w_down, dtype=np.float32)

    xT_b = np.ascontiguousarray(hs.T).astype(bf)
    inv_freq = 1.0 / (THETA ** (np.arange(0, HD, 2, dtype=np.float64) / HD))
    freqs = pos[:, None] * inv_freq[None, :]            # [SEQ, 64]
    cos_h = np.cos(freqs).T.astype(np.float32)          # [64, SEQ]
    sin_h = np.sin(freqs).T.astype(np.float32)
    cos_t = np.ascontiguousarray(np.concatenate([cos_h, cos_h], axis=0))
    sin_t = np.ascontiguousarray(np.concatenate([-sin_h, sin_h], axis=0))

    q_size = NH * HD
    kv_size = NKV * HD
    wq_eff = wq * ln1[:, None]
    wgu_eff = wgu * ln2[:, None]
    scale = HD ** -0.5

    # shared (replicated) weights, pre-tiled to SBUF layout:
    # w_o_f[n][p, a*128+c] = wo[a*128+p, n*128+c]
    w_o_f = np.ascontiguousarray(
        wo.reshape(HB, 128, HB, 128).transpose(2, 1, 0, 3).reshape(HB, 128, HB * 128)
    ).astype(bf)
    gu_parts = []
    for m in range(NM):
        gu_parts.append(wgu_eff[:, m * 128:(m + 1) * 128])
        gu_parts.append(wgu_eff[:, INTER + m * 128:INTER + (m + 1) * 128])
    gu_il = np.concatenate(gu_parts, axis=1)  # [2048, 11264] g/u interleaved
    # w_gu_f[m][p, hb*256+c] = gu_il[hb*128+p, m*256+c]
    w_gu_f = np.ascontiguousarray(
        gu_il.reshape(HB, 128, NM, 256).transpose(2, 1, 0, 3).reshape(NM, 128, HB * 256)
    ).astype(bf)
    # w_dn_f[n][p, m*128+c] = wdn[m*128+p, n*128+c]
    w_dn_f = np.ascontiguousarray(
        wdn.reshape(NM, 128, HB, 128).transpose(2, 1, 0, 3).reshape(HB, 128, NM * 128)
    ).astype(bf)

    per_core = []
    for c in range(NCORES):
        kvh = c // 2
        q_cols = wq_eff[:, 2 * c * HD:(2 * c + ACOLS) * HD] * scale
        k_cols = wq_eff[:, q_size + kvh * HD:q_size + (kvh + 1) * HD]
        v_cols = wq_eff[:, q_size + kv_size + kvh * HD:q_size + kv_size + (kvh + 1) * HD]
        # column layout must match wq_sb rearrange: per hidden-block rows,
        # cols [q0, q1, k, v]
        w_qkv_c = np.ascontiguousarray(
            np.concatenate([q_cols, k_cols, v_cols], axis=1)).astype(bf)
        xT_own = np.ascontiguousarray(hs[c * OWN:(c + 1) * OWN].T)
        per_core.append({
            "xT_b": xT_b, "xT_own": xT_own,
            "cos_t": cos_t, "sin_t": sin_t,
            "w_qkv_s": w_qkv_c, "w_o_f": w_o_f,
            "w_gu_f": w_gu_f, "w_dn_f": w_dn_f,
        })
    return per_core


def kernel(positions, hidden_states, ln1_w, ln2_w, w_qkv, w_o, w_gate_up, w_down):
    runner = _get_runner()
    per_core = _prep_inputs(positions, hidden_states, ln1_w, ln2_w,
                            w_qkv, w_o, w_gate_up, w_down)
    concat_in = [
        np.concatenate([np.asarray(per_core[c][name]) for c in range(NCORES)], axis=0)
        for name in runner["in_names"]
    ]
    concat_zeros = [
        np.zeros((NCORES * s[0],) + tuple(s[1:]), d)
        for (s, d) in runner["zero_shapes"]
    ]
    outs = runner["fn"](*concat_in, *concat_zeros)
    out = np.asarray(outs[0]).reshape(NCORES, HID, OWN)
    # core c owns rows [c*OWN, (c+1)*OWN), transposed
    full = out.transpose(0, 2, 1).reshape(SEQ, HID)
    return full


if __name__ == "__main__":
    print("building...")
    _get_runner()
    print("built ok")

